# revision 14
# baseline (speedup 1.0000x reference)
"""Trainium2 Bass kernel for nn_Criterion_28003186770325.

Contrastive CE loss (keypoint features vs normalized neural mesh memory)
+ background-mask MSE, data-parallel over the batch axis B=8 on 8 cores.

The mesh memory is normalized + pad-masked + transposed on the host
(parameter preprocessing). Per core (one batch element), the softmax
denominator S_r = sum_j exp(kappa*sim_rj - SHIFT) over 1536 vis-packed
kp rows x 12288 mesh columns is computed by a hybrid of two unit
orientations sharing three [128,1024] f32 PSUM slots (6 banks):

  A-units (row-major, cols 0:6144): sim tile = kpT_jt^T @ nmmnT chunk,
    partitions = kp rows. Exp+row-accumulate fused on the Scalar engine
    (ACT Exp + accum_out), or on Vector via a Schraudolph u16 pass with
    a GpSimd halving add and a deferred 512-wide reduce.
  B-units (col-major, cols 6144:12288): sim tile = nmmnT_jtile^T @ kpT,
    partitions = mesh cols. Exp via the Vector u16 pass to SBUF; the
    row-reduction runs on the PE itself as ones^T @ exp accumulated
    into a persistent [1,1024] PSUM accumulator (2 banks) over all 48
    column tiles -- no Scalar reads, no Vector reduces. Two row phases
    (rows 0:1024, then 1024:1536 with paired half-slots) bound the
    accumulator to 1024 entries; each phase ends with a tiny PSUM->SBUF
    extract, and 12 one-column matmuls transpose the accumulated row
    sums back to the per-row-partition layout at the end.

  CE_r = ln(S_r) - (kappa*t_r - SHIFT), ln via bitcast.

Self-contained: hardcodes all shapes; no file reads.
"""

import sys

if "/opt/trn_rl_repo" not in sys.path:
    sys.path.insert(0, "/opt/trn_rl_repo")

import math
import os
from contextlib import ExitStack

import numpy as np

import concourse.bass as bass
import concourse.mybir as mybir
from concourse import bacc
from concourse.bass_utils import run_bass_kernel_spmd
from concourse.tile import TileContext

# problem dims
B, V, D, C, H, W = 8, 1024, 128, 12, 224, 224
CV = C * V                     # 12288
KAPPA = 1.0 / 0.07
N_CORES = 8
P = 128
NPK = 12                       # packed kp row tiles (12*128 = 1536 rows)
NR = NPK * P                   # 1536 packed rows
HW = H * W                     # 50176 = 128*392
BGF = HW // P                  # 392

SHIFT = 96.0

AF = mybir.ActivationFunctionType
OP = mybir.AluOpType
dt = mybir.dt

# Schraudolph exp in bf16 domain: u16 = clamp0(A16*(kappa*sim - SHIFT) + B16)
_f = np.linspace(0.0, 1.0, 1 << 20, endpoint=False) + 0.5 / (1 << 20)
_c = float(np.mean((1.0 + _f) / np.exp2(_f)) - 1.0) / float(
    np.mean(1.0 / np.exp2(_f)))
A16 = 128.0 / math.log(2.0)
B16 = 127.0 * 128.0 - _c * 128.0
# bitcast-ln: ln(x) ~= (bitcast_i32(x)/2^23 - 127 + cln)*ln2
_CLN = float(np.mean(np.log2(1.0 + _f) - _f))
LN_SCALE = math.log(2.0) / 8388608.0
LN_BIAS = -(127.0 - _CLN) * math.log(2.0)

# unit partitioning
NB = 48                        # B-path j-tiles (cols 6144:12288)
ACOLS = CV - NB * P            # 6144 A-path columns
NCH = ACOLS // 1024            # 6 A-chunks per row tile
NDVE_A = int(os.environ.get("KNDVE", "5"))   # A-units on the DVE path
KREP = int(os.environ.get("KREP", "1"))

_compiled = {}


def _build():
    nc = bacc.Bacc("TRN2", target_bir_lowering=False, debug=False,
                   num_devices=N_CORES)

    nmmnT_ext = nc.declare_dram_parameter("nmmnT", [P, 6, 2048],
                                          dt.float16, isOutput=False)
    kpT16_ext = nc.declare_dram_parameter("kpT16", [P, NR], dt.float16,
                                          isOutput=False)
    kp16_ext = nc.declare_dram_parameter("kp16", [P, NPK, D], dt.float16,
                                         isOutput=False)
    selp16_ext = nc.declare_dram_parameter("selp16", [P, NPK, D], dt.float16,
                                           isOutput=False)
    w_ext = nc.declare_dram_parameter("wmi", [P, 2 * NPK], dt.float16,
                                      isOutput=False)
    bg16_ext = nc.declare_dram_parameter("bg16", [P, 4, BGF], dt.float16,
                                         isOutput=False)
    out_ext = nc.declare_dram_parameter("out", [1, 8], dt.float32,
                                        isOutput=True)

    with TileContext(nc) as tc, ExitStack() as ctx:
        consts = ctx.enter_context(tc.tile_pool(name="consts", bufs=1))
        sbig = ctx.enter_context(tc.tile_pool(name="sbig", bufs=1))
        dumps = ctx.enter_context(tc.tile_pool(name="dumps", bufs=4))
        hpool = ctx.enter_context(tc.tile_pool(name="hpool", bufs=4))
        work = ctx.enter_context(tc.tile_pool(name="work", bufs=2))
        # 3 rotating [128,1024] slots (6 banks) + [1,1024] accumulator
        pm = ctx.enter_context(tc.tile_pool(name="pm", bufs=1, space="PSUM"))

        for _rep in range(KREP):
            kpT16 = sbig.tile([P, NR], dt.float16)
            nmmnT = sbig.tile([P, CV], dt.float16)
            nc.sync.dma_start(out=kpT16, in_=kpT16_ext[:])
            # DMA pair-chunks: B-path cols (3..5) race the A-path (0..2)
            for pr in (3, 0, 4, 1, 5, 2):
                nc.sync.dma_start(
                    out=nmmnT[:, pr * 2048:(pr + 1) * 2048],
                    in_=nmmnT_ext.ap()[:, pr])

            ones_col = consts.tile([P, 1], dt.float16)
            nc.vector.memset(ones_col, 1.0)
            ones_bf = consts.tile([P, 1], dt.bfloat16)
            nc.vector.memset(ones_bf, 1.0)
            ones_f1 = consts.tile([1, 1], dt.float32)
            nc.vector.memset(ones_f1, 1.0)
            neg_shift = consts.tile([P, 1], dt.float32)
            nc.vector.memset(neg_shift, -SHIFT)
            adump1 = consts.tile([P, 8], dt.bfloat16)

            # cevblock: [0:12]=cev_m [12:24]=cev_i [24:48]=w_m,w_i [48:50]=bg
            cevblock = sbig.tile([P, 52], dt.float16)
            nc.scalar.dma_start(out=cevblock[:, 24:48], in_=w_ext[:])
            kp16 = sbig.tile([P, NPK, D], dt.float16)
            nc.scalar.dma_start(out=kp16, in_=kp16_ext[:])
            selp16 = sbig.tile([P, NPK, D], dt.float16)
            nc.scalar.dma_start(out=selp16, in_=selp16_ext[:])
            bg16 = sbig.tile([P, 4, BGF], dt.float16)
            nc.scalar.dma_start(out=bg16, in_=bg16_ext[:])

            # ---- persistent state ------------------------------------------
            partials = sbig.tile([P, NPK * NCH], dt.float32)
            paccS = sbig.tile([1, NR], dt.float32)
            bgacc = sbig.tile([P, 2], dt.float32)
            dummy1 = consts.tile([P, 1], dt.float32)
            traw = sbig.tile([P, NPK], dt.float32)

            pmall = pm.tile([P, 3, 1024], dt.float32)
            pacc = pm.tile([1, 1024], dt.float32)

            # ---- unit machinery --------------------------------------------
            slot_i = [0]

            def next_slot():
                s = slot_i[0]
                slot_i[0] = (s + 1) % 3
                return pmall[:, s, :]

            a_cnt = [0]
            pendingV = []             # deferred A-DVE reduces: (h, pidx)
            pendingB = []             # deferred B reduce-MM thunks

            def flush_v():
                h, pidx = pendingV.pop(0)
                nc.vector.tensor_reduce(
                    out=partials[:, pidx:pidx + 1], in_=h,
                    axis=mybir.AxisListType.X, op=OP.add)

            def flush_b():
                pendingB.pop(0)()

            def a_unit(jt, ch):
                # rows jt*128.., cols ch*1024.. : row-major orientation
                pmt = next_slot()
                lhsT = kpT16[:, jt * P:(jt + 1) * P]
                base = ch * 1024
                for k in range(2):
                    nc.tensor.matmul(
                        pmt[:, k * 512:(k + 1) * 512],
                        lhsT=lhsT,
                        rhs=nmmnT[:, base + k * 512: base + (k + 1) * 512],
                        start=True, stop=True)
                pidx = jt * NCH + ch
                u = a_cnt[0]
                a_cnt[0] += 1
                # spread the NDVE_A DVE-path units over the 72 A-units
                is_dve = (u % (72 // max(NDVE_A, 1))) == 2 and \
                    sum(1 for x in range(u) if (x % (72 // max(NDVE_A, 1))) == 2) < NDVE_A
                if not is_dve:
                    dump = adump1[:, 0:1].broadcast_to((P, 1024))
                    nc.scalar.activation(
                        out=dump, in_=pmt, func=AF.Exp,
                        bias=neg_shift[:], scale=KAPPA,
                        accum_out=partials[:, pidx:pidx + 1])
                else:
                    e16 = dumps.tile([P, 1024], dt.uint16, tag="e16")
                    nc.vector.tensor_scalar(
                        out=e16, in0=pmt,
                        scalar1=A16 * KAPPA, scalar2=B16 - A16 * SHIFT,
                        op0=OP.mult, op1=OP.add)
                    eb = e16.bitcast(dt.bfloat16)
                    h = hpool.tile([P, 512], dt.bfloat16, tag="h")
                    nc.gpsimd.tensor_tensor(
                        out=h, in0=eb[:, 0:512], in1=eb[:, 512:1024],
                        op=OP.add)
                    pendingV.append((h, pidx))
                    if len(pendingV) > 2:
                        flush_v()

            bstate = {"p0_first": True, "p1_first": True}

            def b_unit_p0(bi):
                # j-tile NB-range index bi (0..47), rows 0:1024
                pmt = next_slot()
                jcol = ACOLS + bi * P
                lhsT = nmmnT[:, jcol:jcol + P]
                for k in range(2):
                    nc.tensor.matmul(
                        pmt[:, k * 512:(k + 1) * 512],
                        lhsT=lhsT, rhs=kpT16[:, k * 512:(k + 1) * 512],
                        start=True, stop=True)
                e16 = dumps.tile([P, 1024], dt.uint16, tag="e16")
                nc.vector.tensor_scalar(
                    out=e16, in0=pmt,
                    scalar1=A16 * KAPPA, scalar2=B16 - A16 * SHIFT,
                    op0=OP.mult, op1=OP.add)
                eb = e16.bitcast(dt.bfloat16)
                first = bstate["p0_first"]
                bstate["p0_first"] = False

                def red(eb=eb, first=first):
                    for k in range(2):
                        nc.tensor.matmul(
                            pacc[:, k * 512:(k + 1) * 512],
                            lhsT=ones_bf[:],
                            rhs=eb[:, k * 512:(k + 1) * 512],
                            start=first, stop=(bi == NB - 1),
                            skip_group_check=True)
                pendingB.append(red)
                if len(pendingB) > 1:
                    flush_b()

            def b_unit_p1(biA, biB):
                # paired j-tiles, rows 1024:1536 (512 each half-slot)
                pmt = next_slot()
                for half, bi in ((0, biA), (1, biB)):
                    jcol = ACOLS + bi * P
                    nc.tensor.matmul(
                        pmt[:, half * 512:(half + 1) * 512],
                        lhsT=nmmnT[:, jcol:jcol + P],
                        rhs=kpT16[:, 1024:1536],
                        start=True, stop=True)
                e16 = dumps.tile([P, 1024], dt.uint16, tag="e16")
                nc.vector.tensor_scalar(
                    out=e16, in0=pmt,
                    scalar1=A16 * KAPPA, scalar2=B16 - A16 * SHIFT,
                    op0=OP.mult, op1=OP.add)
                eb = e16.bitcast(dt.bfloat16)
                first = bstate["p1_first"]
                bstate["p1_first"] = False

                def red(eb=eb, first=first, last=(biB == NB - 1)):
                    for k in range(2):
                        nc.tensor.matmul(
                            pacc[:, 0:512],
                            lhsT=ones_bf[:],
                            rhs=eb[:, k * 512:(k + 1) * 512],
                            start=(first and k == 0), stop=(last and k == 1),
                            skip_group_check=True)
                pendingB.append(red)
                if len(pendingB) > 1:
                    flush_b()

            def bg_mse():
                diffs = sbig.tile([P, 2, BGF], dt.float16)
                for s in range(2):
                    nc.gpsimd.tensor_tensor(
                        out=diffs[:, s, :], in0=bg16[:, s, :],
                        in1=bg16[:, 2 + s, :], op=OP.subtract)
                    d2 = work.tile([P, BGF], dt.float16, tag="d2")
                    nc.gpsimd.tensor_tensor(out=d2, in0=diffs[:, s, :],
                                            in1=diffs[:, s, :], op=OP.mult)
                    nc.vector.tensor_scalar(
                        out=dummy1.broadcast_to((P, BGF)),
                        in0=d2, scalar1=1.0, scalar2=0.0,
                        op0=OP.mult, op1=OP.add,
                        accum_out=bgacc[:, s:s + 1])

            def traw_work():
                q = work.tile([P, NPK * D], dt.float16, tag="q")
                nc.gpsimd.tensor_tensor(
                    out=q, in0=kp16.rearrange("p t d -> p (t d)"),
                    in1=selp16.rearrange("p t d -> p (t d)"), op=OP.mult)
                nc.vector.tensor_reduce(
                    out=traw, in_=q.rearrange("p (t d) -> p t d", t=NPK),
                    axis=mybir.AxisListType.X, op=OP.add)

            # ---- main schedule ---------------------------------------------
            # segment 1: 48 rounds of (A-unit, B-phase0); A-units iterate
            # chunk-major so early units only need DMA pair-chunk 0.
            a_list = [(jt, ch) for ch in range(NCH) for jt in range(NPK)]
            ai = 0
            for r in range(NB):
                jt, ch = a_list[ai]
                ai += 1
                a_unit(jt, ch)
                b_unit_p0(r)
                if r == 20:
                    bg_mse()
                if r == 30:
                    traw_work()
            while pendingB:
                flush_b()
            # extract phase-0 row sums (rows 0:1024)
            nc.scalar.copy(out=paccS[:, 0:1024], in_=pacc[:, 0:1024])

            # segment 2: 24 rounds of (A-unit, B-phase1 pair)
            for r in range(NB // 2):
                jt, ch = a_list[ai]
                ai += 1
                a_unit(jt, ch)
                b_unit_p1(2 * r, 2 * r + 1)
            while pendingB:
                flush_b()
            while pendingV:
                flush_v()
            nc.scalar.copy(out=paccS[:, 1024:1536], in_=pacc[:, 0:512])

            # ---- transpose B row sums into per-row-partition layout ---------
            sb = pmall[:, 2, 0:NPK]
            for jt in range(NPK):
                nc.tensor.matmul(
                    sb[:, jt:jt + 1],
                    lhsT=paccS[:, jt * P:(jt + 1) * P],
                    rhs=ones_f1[:], start=True, stop=True)

            # ---- finalize ---------------------------------------------------
            S = sbig.tile([P, NPK], dt.float32)
            nc.vector.tensor_reduce(
                out=S,
                in_=partials.rearrange("p (a k) -> p a k", k=NCH),
                axis=mybir.AxisListType.X, op=OP.add)
            nc.vector.tensor_tensor(out=S, in0=S, in1=sb, op=OP.add)
            lse = sbig.tile([P, NPK], dt.float32)
            nc.vector.tensor_scalar(
                out=lse, in0=S.bitcast(dt.int32), scalar1=LN_SCALE,
                scalar2=LN_BIAS, op0=OP.mult, op1=OP.add)

            # tnorm = kappa * traw - SHIFT ; ce = lse - tnorm
            tnorm = sbig.tile([P, NPK], dt.float32)
            nc.vector.tensor_scalar(
                out=tnorm, in0=traw, scalar1=KAPPA, scalar2=-SHIFT,
                op0=OP.mult, op1=OP.add)
            ce = sbig.tile([P, NPK], dt.float16)
            nc.vector.tensor_tensor(out=ce, in0=lse, in1=tnorm,
                                    op=OP.subtract)
            nc.vector.tensor_mul(cevblock[:, 0:NPK], ce,
                                 cevblock[:, 24:24 + NPK])
            nc.vector.tensor_mul(cevblock[:, NPK:2 * NPK], ce,
                                 cevblock[:, 24 + NPK:24 + 2 * NPK])
            nc.vector.tensor_copy(out=cevblock[:, 48:50], in_=bgacc)

            # ---- partition reduction via ones-matmul ------------------------
            fin = pmall[0:1, 0, 0:50]
            nc.tensor.matmul(fin, lhsT=ones_col[:],
                             rhs=cevblock[:, 0:50], start=True, stop=True)
            outv = sbig.tile([1, 8], dt.float32)
            nc.vector.tensor_reduce(
                out=outv[:, 0:4],
                in_=fin[:, 0:48].rearrange("q (a t) -> q a t", t=NPK),
                axis=mybir.AxisListType.X, op=OP.add)
            nc.vector.tensor_copy(out=outv[:, 4:6], in_=fin[:, 48:50])
            nc.vector.tensor_copy(out=outv[:, 6:7], in_=lse[0:1, 0:1])
            nc.vector.tensor_copy(out=outv[:, 7:8], in_=tnorm[0:1, 0:1])
            nc.sync.dma_start(out=out_ext[:], in_=outv)

    nc.finalize()
    return nc


def _get_nc():
    if "nc" not in _compiled:
        _compiled["nc"] = _build()
    return _compiled["nc"]


def kernel(kp_feats_m, kp_feats_i, label, kp_vis_m, kp_vis_i,
           neural_mesh_memory, pad_index, bg_m, bg_i, mask_gt_m, mask_gt_i,
           _want_results=False, _trace=False):
    nc = _get_nc()

    kp_m = np.asarray(kp_feats_m, dtype=np.float32)
    kp_i = np.asarray(kp_feats_i, dtype=np.float32)
    nmm = np.asarray(neural_mesh_memory, dtype=np.float32)
    lab = np.asarray(label).astype(np.int64).reshape(B)
    vis_m = np.asarray(kp_vis_m).astype(bool)
    vis_i = np.asarray(kp_vis_i).astype(bool)
    pad = np.asarray(pad_index).astype(bool)
    bgs = [np.asarray(a, dtype=np.float32).reshape(B, HW)
           for a in (bg_m, bg_i, mask_gt_m, mask_gt_i)]

    # normalized + pad-masked mesh memory (parameter preprocessing)
    nmmn = nmm / np.maximum(
        np.linalg.norm(nmm, axis=-1, keepdims=True), 1e-30)
    nmmn = nmmn * (~pad)[..., None]
    nmmnT16 = np.ascontiguousarray(
        nmmn.reshape(CV, D).T.astype(np.float16))           # (128, 12288)

    def pack_pf(a_rows):          # (NR, k) -> (P, NPK, k) row-tile layout
        return np.ascontiguousarray(
            a_rows.reshape(NPK, P, -1).transpose(1, 0, 2).astype(np.float16))

    in_maps = []
    for b in range(B):
        allv = np.concatenate([vis_m[b], vis_i[b]])            # (2048,)
        order = np.argsort(~allv, kind="stable")[:NR]
        kp_all = np.concatenate([kp_m[b], kp_i[b]])            # (2048, D)
        kpp = kp_all[order]                                    # (NR, D)
        vertex = order % V
        setid = order // V
        w = allv[order].astype(np.float16)
        w_m = (w * (setid == 0)).astype(np.float16)
        w_i = (w * (setid == 1)).astype(np.float16)
        wmi = np.ascontiguousarray(np.concatenate([
            w_m.reshape(NPK, P).T, w_i.reshape(NPK, P).T],
            axis=1))                                           # (P, 2*NPK)
        kpT16 = np.ascontiguousarray(kpp.T.astype(np.float16))  # (D, NR)
        bg16 = np.ascontiguousarray(
            np.stack([a[b] for a in bgs]).reshape(4, P, BGF)
            .transpose(1, 0, 2).astype(np.float16))
        in_maps.append({
            "nmmnT": nmmnT16,
            "kpT16": kpT16,
            "kp16": pack_pf(kpp),
            "selp16": pack_pf(nmmn[lab[b]][vertex]),
            "wmi": wmi,
            "bg16": bg16,
        })

    res = run_bass_kernel_spmd(nc, in_maps, list(range(N_CORES)),
                               trace=_trace)
    outs = np.stack([res.results[b]["out"][0] for b in range(B)])  # (8, 8)

    ce_m, ce_i = outs[:, 0].sum(), outs[:, 1].sum()
    vm, vi = outs[:, 2].sum(), outs[:, 3].sum()
    sse_m, sse_i = outs[:, 4].sum(), outs[:, 5].sum()
    loss = 0.5 * (ce_m / vm + ce_i / vi)
    mask_loss = 0.5 * (sse_m + sse_i) / HW / B
    result = np.array([loss, mask_loss], dtype=np.float32)
    if _want_results:
        return result, res, outs
    return result
